# revision 4
# baseline (speedup 1.0000x reference)
"""Distributed Trainium2 Bass kernel for sparse coor_descent attention.

Strategy: one head per NeuronCore (8 heads / 8 cores).
Key algebraic reformulation of coor_descent (k=1, constant=0):
    s+b = min(s, -a)  and exp is monotone, so with S = s/eps, eS = exp(S):
        r_{t} = sum_j min(eS_ij, r_{t-1,i}),   r_0 = 1
        attn  = min(eS / r_25, 1)
which runs as ONE fused DVE tensor_scalar(min, accum_out=sum) per row-tile
per iteration -- no transcendentals in the loop.

LN affine (gamma/beta) is folded into w_qkv on the host; the q scale and
the 1/eps are folded into the q-projection weights. Causal masking zeroes
eS above the diagonal (exp(-inf) = 0); strictly-upper blocks are never
stored or processed (triangular work-skipping).

Final projection: per-head output columns are exchanged via AllToAll so
core c ends with all heads' outputs for its token block, then computes
y rows [128c:128c+128] = outT_all^T @ w_out locally.
"""

import sys
import numpy as np

sys.path.insert(0, "/opt/trn_rl_repo")

HEADS = 8
DH = 64
DIM = 512
N = 1024
P = 128
NT = N // P  # 8 token row-tiles
KC = DIM // P  # 4 contraction chunks
EPS = 0.1
LN_EPS = 1e-5
N_ITERS = 25
QSCALE = (DH ** -0.5) / EPS  # fold head scale and 1/eps into q

_cache = {}


def _build():
    from concourse import bacc, mybir
    import concourse.bass as bass
    import concourse.tile as tile
    from concourse.masks import make_identity

    f32 = mybir.dt.float32
    Alu = mybir.AluOpType
    Act = mybir.ActivationFunctionType

    nc = bacc.Bacc("TRN2", target_bir_lowering=False, debug=False,
                   enable_asserts=True, num_devices=HEADS)

    x_ext = nc.dram_tensor("x", [N, DIM], f32, kind="ExternalInput")
    wq_ext = nc.dram_tensor("wq", [DIM, DH], f32, kind="ExternalInput")
    wk_ext = nc.dram_tensor("wk", [DIM, DH], f32, kind="ExternalInput")
    wv_ext = nc.dram_tensor("wv", [DIM, DH], f32, kind="ExternalInput")
    bq_ext = nc.dram_tensor("bq", [DH, 1], f32, kind="ExternalInput")
    bk_ext = nc.dram_tensor("bk", [DH, 1], f32, kind="ExternalInput")
    bv_ext = nc.dram_tensor("bv", [1, DH], f32, kind="ExternalInput")
    wo_ext = nc.dram_tensor("wo", [DIM, DIM], f32, kind="ExternalInput")
    out_ext = nc.dram_tensor("out", [P, DIM], f32, kind="ExternalOutput")

    with tile.TileContext(nc) as tc:
        with (
            tc.tile_pool(name="sb", bufs=1) as sb,
            tc.tile_pool(name="pmm", bufs=3, space="PSUM") as pmm,
            tc.tile_pool(name="pqk", bufs=2, space="PSUM") as pqk,
            tc.tile_pool(name="ptr", bufs=3, space="PSUM") as ptr,
            tc.tile_pool(name="dram", bufs=1, space="DRAM") as dram,
        ):
            ident = sb.tile([P, P], f32, tag="ident")
            make_identity(nc, ident[:])

            # ---- weight DMAs ----
            wq_sb = sb.tile([P, KC, DH], f32, tag="wq")
            wk_sb = sb.tile([P, KC, DH], f32, tag="wk")
            wv_sb = sb.tile([P, KC, DH], f32, tag="wv")
            nc.sync.dma_start(wq_sb[:], wq_ext[:].rearrange("(kc p) m -> p kc m", p=P))
            nc.sync.dma_start(wk_sb[:], wk_ext[:].rearrange("(kc p) m -> p kc m", p=P))
            nc.sync.dma_start(wv_sb[:], wv_ext[:].rearrange("(kc p) m -> p kc m", p=P))
            bq_sb = sb.tile([DH, 1], f32, tag="bq")
            bk_sb = sb.tile([DH, 1], f32, tag="bk")
            bv_sb = sb.tile([1, DH], f32, tag="bv")
            nc.sync.dma_start(bq_sb[:], bq_ext[:])
            nc.sync.dma_start(bk_sb[:], bk_ext[:])
            nc.sync.dma_start(bv_sb[:], bv_ext[:])
            wo_sb = sb.tile([P, KC, DIM], f32, tag="wo")
            nc.sync.dma_start(wo_sb[:], wo_ext[:].rearrange("(kc p) e -> p kc e", p=P))
            ones_sb = sb.tile([1, P], f32, tag="ones")
            nc.vector.memset(ones_sb[:], 1.0)

            # ---- x DMA + LayerNorm (no affine; folded into weights) ----
            xin = [sb.tile([P, DIM], f32, tag=f"xin{t}", name=f"xin{t}") for t in range(NT)]
            xh = [sb.tile([P, DIM], f32, tag=f"xh{t}", name=f"xh{t}") for t in range(NT)]
            sq_scr = sb.tile([P, DIM], f32, tag="sq_scr")
            for t in range(NT):
                nc.sync.dma_start(xin[t][:], x_ext[P * t:P * (t + 1), :])
            for t in range(NT):
                stat = sb.tile([P, 6], f32, tag=f"stat{t}")
                # stat cols: 0=sum, 1=ssq, 2=mu, 3=bias(eps-mu^2), 4=std, 5=rstd
                nc.vector.tensor_reduce(stat[:, 0:1], xin[t][:], mybir.AxisListType.X, Alu.add)
                nc.scalar.activation(sq_scr[:], xin[t][:], Act.Square, accum_out=stat[:, 1:2])
                nc.scalar.mul(stat[:, 2:3], stat[:, 0:1], 1.0 / DIM)
                nc.scalar.square(stat[:, 3:4], stat[:, 2:3])
                nc.vector.tensor_scalar(stat[:, 3:4], stat[:, 3:4], -1.0, LN_EPS, Alu.mult, Alu.add)
                nc.scalar.activation(stat[:, 4:5], stat[:, 1:2], Act.Sqrt,
                                     bias=stat[:, 3:4], scale=1.0 / DIM)
                nc.vector.reciprocal(stat[:, 5:6], stat[:, 4:5])
                nc.vector.tensor_scalar(xh[t][:], xin[t][:], stat[:, 2:3], stat[:, 5:6],
                                        Alu.subtract, Alu.mult)

            # ---- transpose xh -> xhT [512, 1024] (4 tiles of [128, 1024]) ----
            xhT = [sb.tile([P, N], f32, tag=f"xhT{u}", name=f"xhT{u}") for u in range(KC)]
            for t in range(NT):
                for u in range(KC):
                    tr = ptr.tile([P, P], f32, tag="tr")
                    nc.tensor.transpose(tr[:], xh[t][:, P * u:P * (u + 1)], ident[:])
                    if (t + u) % 2 == 0:
                        nc.scalar.copy(xhT[u][:, P * t:P * (t + 1)], tr[:])
                    else:
                        nc.vector.tensor_copy(xhT[u][:, P * t:P * (t + 1)], tr[:])

            # ---- qT/kT = [64, 1024], v natural [128, 64] x 8 ----
            qT = sb.tile([DH, N], f32, tag="qT")
            kT = sb.tile([DH, N], f32, tag="kT")
            for dst_sb, w_sb, b_sb in ((qT, wq_sb, bq_sb), (kT, wk_sb, bk_sb)):
                for nb in range(2):
                    ps = pqk.tile([DH, 512], f32, tag="pqk")
                    for kc in range(KC):
                        nc.tensor.matmul(ps[:], w_sb[:, kc, :],
                                         xhT[kc][:, 512 * nb:512 * (nb + 1)],
                                         start=(kc == 0), stop=(kc == KC - 1))
                    nc.scalar.activation(dst_sb[:, 512 * nb:512 * (nb + 1)], ps[:],
                                         Act.Identity, bias=b_sb[:])
            v_sb = [sb.tile([P, DH], f32, tag=f"v{c}", name=f"v{c}") for c in range(NT)]
            for c in range(NT):
                ps = pqk.tile([P, DH], f32, tag="pqk")
                for kc in range(KC):
                    nc.tensor.matmul(ps[:], xhT[kc][:, P * c:P * (c + 1)], wv_sb[:, kc, :],
                                     start=(kc == 0), stop=False)
                nc.tensor.matmul(ps[:], ones_sb[:, 0:P], bv_sb[:], start=False, stop=True)
                if c % 2 == 0:
                    nc.scalar.copy(v_sb[c][:], ps[:])
                else:
                    nc.vector.tensor_copy(v_sb[c][:], ps[:])

            # ---- sim matmuls + fused exp: eS[m] = exp(qT_m^T @ kT), causal ----
            eS = [sb.tile([P, P * (m + 1)], f32, tag=f"eS{m}", name=f"eS{m}") for m in range(NT)]
            for m in range(NT):
                W = P * (m + 1)
                for nb in range((W + 511) // 512):
                    w = min(512, W - 512 * nb)
                    ps = pmm.tile([P, 512], f32, tag="psim")
                    nc.tensor.matmul(ps[:, :w], qT[:, P * m:P * (m + 1)],
                                     kT[:, 512 * nb:512 * nb + w])
                    nc.scalar.activation(eS[m][:, 512 * nb:512 * nb + w], ps[:, :w], Act.Exp)
                # causal mask on the diagonal block: keep j <= p, else 0
                nc.gpsimd.affine_select(
                    out=eS[m][:, W - P:W], in_=eS[m][:, W - P:W],
                    compare_op=Alu.is_ge, fill=0.0, base=0,
                    pattern=[[-1, P]], channel_multiplier=1)

            # ---- the coor_descent loop: r_t = sum_j min(eS, r_{t-1}) ----
            es = [sb.tile([P, P * (m + 1)], f32, tag=f"es{m}", name=f"es{m}") for m in range(NT)]
            r = [sb.tile([P, N_ITERS + 1], f32, tag=f"r{m}", name=f"r{m}") for m in range(NT)]
            for m in range(NT):
                nc.vector.memset(r[m][:, 0:1], 1.0)
            for it in range(1, N_ITERS + 1):
                for m in range(NT):
                    W = P * (m + 1)
                    nc.vector.tensor_scalar(
                        es[m][:, :W], eS[m][:, :W], r[m][:, it - 1:it], None,
                        Alu.min, Alu.add, accum_out=r[m][:, it:it + 1])

            # ---- final attn = min(eS * (1/r), 1), in es scratch ----
            for m in range(NT):
                W = P * (m + 1)
                rec = sb.tile([P, 1], f32, tag=f"rec{m}")
                nc.vector.reciprocal(rec[:], r[m][:, N_ITERS:N_ITERS + 1])
                nc.vector.tensor_scalar(es[m][:, :W], eS[m][:, :W], rec[:], 1.0,
                                        Alu.mult, Alu.min)

            # ---- transpose attn -> attnT per j-chunk c: [128, N - 128c] ----
            aT = [sb.tile([P, N - P * c], f32, tag=f"aT{c}", name=f"aT{c}") for c in range(NT)]
            for m in range(NT):
                for c in range(m + 1):
                    tr = ptr.tile([P, P], f32, tag="tr")
                    nc.tensor.transpose(tr[:], es[m][:, P * c:P * (c + 1)], ident[:])
                    dst = aT[c][:, P * (m - c):P * (m - c + 1)]
                    if (m + c) % 2 == 0:
                        nc.scalar.copy(dst, tr[:])
                    else:
                        nc.vector.tensor_copy(dst, tr[:])

            # ---- outT_h[:, m-block] = sum_c v_c^T-contracted attnT -> [64, 8, 128] ----
            oT = sb.tile([DH, NT, P], f32, tag="oT")
            for m in range(NT):
                ps = pqk.tile([DH, P], f32, tag="pqk")
                for c in range(m + 1):
                    nc.tensor.matmul(ps[:], v_sb[c][:], aT[c][:, P * (m - c):P * (m - c + 1)],
                                     start=(c == 0), stop=(c == m))
                if m % 2 == 0:
                    nc.scalar.copy(oT[:, m, :], ps[:])
                else:
                    nc.vector.tensor_copy(oT[:, m, :], ps[:])

            # ---- AllToAll: shard j of core c = outT_c[:, 128j:128j+128] ----
            a2a_in = dram.tile([NT, DH, P], f32, tag="a2a_in")
            a2a_out = dram.tile([NT, DH, P], f32, tag="a2a_out")
            nc.sync.dma_start(a2a_in[:].rearrange("j p f -> p j f"), oT[:])
            nc.gpsimd.collective_compute(
                "AllToAll", Alu.bypass,
                replica_groups=[list(range(HEADS))],
                ins=[a2a_in.opt()], outs=[a2a_out.opt()])

            # ---- y rows for my token block: lhsT = outT_all [512, 128] ----
            oAll = sb.tile([P, KC, P], f32, tag="oAll")
            nc.sync.dma_start(oAll[:], a2a_out[:].rearrange("(kc g) p f -> (g p) kc f", g=2))
            yps = pmm.tile([P, DIM], f32, tag="psim")
            for kc in range(KC):
                nc.tensor.matmul(yps[:], oAll[:, kc, :], wo_sb[:, kc, :],
                                 start=(kc == 0), stop=(kc == KC - 1))
            y_sb = sb.tile([P, DIM], f32, tag="y")
            nc.scalar.copy(y_sb[:], yps[:])
            nc.sync.dma_start(out_ext[:], y_sb[:])

    nc.compile()
    return nc


def _prep_inputs(x, gamma, beta, w_qkv, w_out):
    x2 = np.ascontiguousarray(np.asarray(x, dtype=np.float32).reshape(N, DIM))
    gamma = np.asarray(gamma, dtype=np.float32)
    beta = np.asarray(beta, dtype=np.float32)
    w_qkv = np.asarray(w_qkv, dtype=np.float32)
    w_out = np.ascontiguousarray(np.asarray(w_out, dtype=np.float32))
    wfold = gamma[:, None] * w_qkv          # LN gamma folded into weights
    bfold = beta @ w_qkv                    # LN beta folded into bias
    in_maps = []
    for c in range(HEADS):
        qs = slice(c * DH, (c + 1) * DH)
        ks = slice(DIM + c * DH, DIM + (c + 1) * DH)
        vs = slice(2 * DIM + c * DH, 2 * DIM + (c + 1) * DH)
        in_maps.append({
            "x": x2,
            "wq": np.ascontiguousarray(wfold[:, qs] * QSCALE),
            "wk": np.ascontiguousarray(wfold[:, ks]),
            "wv": np.ascontiguousarray(wfold[:, vs]),
            "bq": np.ascontiguousarray((bfold[qs] * QSCALE)[:, None]),
            "bk": np.ascontiguousarray(bfold[ks][:, None]),
            "bv": np.ascontiguousarray(bfold[vs][None, :]),
            "wo": w_out,
        })
    return in_maps


def kernel(x, gamma, beta, w_qkv, w_out, _trace=False, **trace_kwargs):
    from concourse.bass_utils import run_bass_kernel_spmd

    if "nc" not in _cache:
        _cache["nc"] = _build()
    nc = _cache["nc"]
    in_maps = _prep_inputs(x, gamma, beta, w_qkv, w_out)
    res = run_bass_kernel_spmd(nc, in_maps, core_ids=list(range(HEADS)),
                               trace=_trace, **trace_kwargs)
    if _trace:
        _cache["last_result"] = res
    y = np.concatenate([res.results[c]["out"] for c in range(HEADS)], axis=0)
    return y.reshape(1, N, DIM)


# revision 5
# speedup vs baseline: 1.1359x; 1.1359x over previous
"""Distributed Trainium2 Bass kernel for sparse coor_descent attention.

Strategy: one head per NeuronCore (8 heads / 8 cores).
Key algebraic reformulation of coor_descent (k=1, constant=0):
    s+b = min(s, -a)  and exp is monotone, so with S = s/eps, eS = exp(S):
        r_{t} = sum_j min(eS_ij, r_{t-1,i}),   r_0 = 1
        attn  = min(eS / r_25, 1)
which runs as ONE fused DVE tensor_scalar(min, accum_out=sum) per row-tile
per iteration -- no transcendentals in the loop.

LN affine (gamma/beta) is folded into w_qkv on the host; the q scale and
the 1/eps are folded into the q-projection weights. Causal masking zeroes
eS above the diagonal (exp(-inf) = 0); strictly-upper blocks are never
stored or processed (triangular work-skipping).

Data path is bf16 (activations, weights, eS, attn); all accumulation
(PSUM, the r sums) stays f32. Validated rel err ~7e-3 vs the f32
reference (gate 2e-2).

Final projection: per-head output columns are exchanged via AllToAll so
core c ends with all heads' outputs for its token block, then computes
y rows [128c:128c+128] = outT_all^T @ w_out locally.
"""

import sys
import numpy as np

sys.path.insert(0, "/opt/trn_rl_repo")

HEADS = 8
DH = 64
DIM = 512
N = 1024
P = 128
NT = N // P  # 8 token row-tiles
KC = DIM // P  # 4 contraction chunks
EPS = 0.1
LN_EPS = 1e-5
N_ITERS = 25
QSCALE = (DH ** -0.5) / EPS  # fold head scale and 1/eps into q

# row-tiles of the coor_descent loop handled by GpSimd instead of DVE
GP_TILES = ()

_cache = {}


def _build():
    from concourse import bacc, mybir
    import concourse.bass as bass
    import concourse.tile as tile
    from concourse.masks import make_identity

    f32 = mybir.dt.float32
    bf = mybir.dt.bfloat16
    Alu = mybir.AluOpType
    Act = mybir.ActivationFunctionType

    nc = bacc.Bacc("TRN2", target_bir_lowering=False, debug=False,
                   enable_asserts=True, num_devices=HEADS)

    x_ext = nc.dram_tensor("x", [N, DIM], f32, kind="ExternalInput")
    wq_ext = nc.dram_tensor("wq", [DIM, DH], f32, kind="ExternalInput")
    wk_ext = nc.dram_tensor("wk", [DIM, DH], f32, kind="ExternalInput")
    wv_ext = nc.dram_tensor("wv", [DIM, DH], f32, kind="ExternalInput")
    bq_ext = nc.dram_tensor("bq", [DH, 1], f32, kind="ExternalInput")
    bk_ext = nc.dram_tensor("bk", [DH, 1], f32, kind="ExternalInput")
    bv_ext = nc.dram_tensor("bv", [1, DH], f32, kind="ExternalInput")
    wo_ext = nc.dram_tensor("wo", [DIM, DIM], f32, kind="ExternalInput")
    out_ext = nc.dram_tensor("out", [P, DIM], f32, kind="ExternalOutput")

    with tile.TileContext(nc) as tc:
        with (
            tc.tile_pool(name="sb", bufs=1) as sb,
            tc.tile_pool(name="pmm", bufs=3, space="PSUM") as pmm,
            tc.tile_pool(name="pqk", bufs=2, space="PSUM") as pqk,
            tc.tile_pool(name="ptr", bufs=3, space="PSUM") as ptr,
            tc.tile_pool(name="dram", bufs=1, space="DRAM") as dram,
        ):
            ident = sb.tile([P, P], bf, tag="ident")
            make_identity(nc, ident[:])

            # ---- weight DMAs (f32) + on-chip converts to bf16 ----
            wq_f = sb.tile([P, KC, DH], f32, tag="wq_f")
            wk_f = sb.tile([P, KC, DH], f32, tag="wk_f")
            wv_f = sb.tile([P, KC, DH], f32, tag="wv_f")
            nc.sync.dma_start(wq_f[:], wq_ext[:].rearrange("(kc p) m -> p kc m", p=P))
            nc.sync.dma_start(wk_f[:], wk_ext[:].rearrange("(kc p) m -> p kc m", p=P))
            nc.sync.dma_start(wv_f[:], wv_ext[:].rearrange("(kc p) m -> p kc m", p=P))
            wq_sb = sb.tile([P, KC, DH], bf, tag="wq")
            wk_sb = sb.tile([P, KC, DH], bf, tag="wk")
            wv_sb = sb.tile([P, KC, DH], bf, tag="wv")
            nc.scalar.copy(wq_sb[:], wq_f[:])
            nc.scalar.copy(wk_sb[:], wk_f[:])
            nc.scalar.copy(wv_sb[:], wv_f[:])
            bq_sb = sb.tile([DH, 1], f32, tag="bq")
            bk_sb = sb.tile([DH, 1], f32, tag="bk")
            bv_f = sb.tile([1, DH], f32, tag="bv_f")
            nc.sync.dma_start(bq_sb[:], bq_ext[:])
            nc.sync.dma_start(bk_sb[:], bk_ext[:])
            nc.sync.dma_start(bv_f[:], bv_ext[:])
            bv_sb = sb.tile([1, DH], bf, tag="bv")
            nc.scalar.copy(bv_sb[:], bv_f[:])
            wo_f = sb.tile([P, KC, DIM], f32, tag="wo_f")
            nc.sync.dma_start(wo_f[:], wo_ext[:].rearrange("(kc p) e -> p kc e", p=P))
            wo_sb = sb.tile([P, KC, DIM], bf, tag="wo")
            nc.vector.tensor_copy(wo_sb[:], wo_f[:])
            ones_sb = sb.tile([1, P], bf, tag="ones")
            nc.vector.memset(ones_sb[:], 1.0)

            # ---- x DMA + LayerNorm (no affine; folded into weights) ----
            xin = [sb.tile([P, DIM], f32, tag=f"xin{t}", name=f"xin{t}") for t in range(NT)]
            xh = [sb.tile([P, DIM], bf, tag=f"xh{t}", name=f"xh{t}") for t in range(NT)]
            sq_scr = sb.tile([P, DIM], f32, tag="sq_scr")
            for t in range(NT):
                nc.sync.dma_start(xin[t][:], x_ext[P * t:P * (t + 1), :])
            for t in range(NT):
                stat = sb.tile([P, 6], f32, tag=f"stat{t}", name=f"stat{t}")
                # stat cols: 0=sum, 1=ssq, 2=mu, 3=bias(eps-mu^2), 4=std, 5=rstd
                nc.vector.tensor_reduce(stat[:, 0:1], xin[t][:], mybir.AxisListType.X, Alu.add)
                nc.scalar.activation(sq_scr[:], xin[t][:], Act.Square, accum_out=stat[:, 1:2])
                nc.scalar.mul(stat[:, 2:3], stat[:, 0:1], 1.0 / DIM)
                nc.scalar.square(stat[:, 3:4], stat[:, 2:3])
                nc.vector.tensor_scalar(stat[:, 3:4], stat[:, 3:4], -1.0, LN_EPS, Alu.mult, Alu.add)
                nc.scalar.activation(stat[:, 4:5], stat[:, 1:2], Act.Sqrt,
                                     bias=stat[:, 3:4], scale=1.0 / DIM)
                nc.vector.reciprocal(stat[:, 5:6], stat[:, 4:5])
                nc.vector.tensor_scalar(xh[t][:], xin[t][:], stat[:, 2:3], stat[:, 5:6],
                                        Alu.subtract, Alu.mult)

            # ---- transpose xh -> xhT [512, 1024] (4 tiles of [128, 1024]) ----
            xhT = [sb.tile([P, N], bf, tag=f"xhT{u}", name=f"xhT{u}") for u in range(KC)]
            for t in range(NT):
                for u in range(KC):
                    tr = ptr.tile([P, P], bf, tag="tr")
                    nc.tensor.transpose(tr[:], xh[t][:, P * u:P * (u + 1)], ident[:])
                    if (t + u) % 2 == 0:
                        nc.scalar.copy(xhT[u][:, P * t:P * (t + 1)], tr[:])
                    else:
                        nc.vector.tensor_copy(xhT[u][:, P * t:P * (t + 1)], tr[:])

            # ---- qT/kT = [64, 1024] bf16, v natural [128, 64] x 8 bf16 ----
            qT = sb.tile([DH, N], bf, tag="qT")
            kT = sb.tile([DH, N], bf, tag="kT")
            for dst_sb, w_sb, b_sb in ((qT, wq_sb, bq_sb), (kT, wk_sb, bk_sb)):
                for nb in range(2):
                    ps = pqk.tile([DH, 512], f32, tag="pqk")
                    for kc in range(KC):
                        nc.tensor.matmul(ps[:], w_sb[:, kc, :],
                                         xhT[kc][:, 512 * nb:512 * (nb + 1)],
                                         start=(kc == 0), stop=(kc == KC - 1))
                    nc.scalar.activation(dst_sb[:, 512 * nb:512 * (nb + 1)], ps[:],
                                         Act.Identity, bias=b_sb[:])
            v_sb = [sb.tile([P, DH], bf, tag=f"v{c}", name=f"v{c}") for c in range(NT)]
            for c in range(NT):
                ps = pqk.tile([P, DH], f32, tag="pqk")
                for kc in range(KC):
                    nc.tensor.matmul(ps[:], xhT[kc][:, P * c:P * (c + 1)], wv_sb[:, kc, :],
                                     start=(kc == 0), stop=False)
                nc.tensor.matmul(ps[:], ones_sb[:, 0:P], bv_sb[:], start=False, stop=True)
                if c % 2 == 0:
                    nc.scalar.copy(v_sb[c][:], ps[:])
                else:
                    nc.vector.tensor_copy(v_sb[c][:], ps[:])

            # ---- sim matmuls + fused exp: eS[m] = exp(qT_m^T @ kT), causal ----
            eS = [sb.tile([P, P * (m + 1)], bf, tag=f"eS{m}", name=f"eS{m}") for m in range(NT)]
            for m in range(NT):
                W = P * (m + 1)
                for nb in range((W + 511) // 512):
                    w = min(512, W - 512 * nb)
                    ps = pmm.tile([P, 512], f32, tag="psim")
                    nc.tensor.matmul(ps[:, :w], qT[:, P * m:P * (m + 1)],
                                     kT[:, 512 * nb:512 * nb + w])
                    nc.scalar.activation(eS[m][:, 512 * nb:512 * nb + w], ps[:, :w], Act.Exp)
                # causal mask on the diagonal block: keep j <= p, else 0
                nc.gpsimd.affine_select(
                    out=eS[m][:, W - P:W], in_=eS[m][:, W - P:W],
                    compare_op=Alu.is_ge, fill=0.0, base=0,
                    pattern=[[-1, P]], channel_multiplier=1)

            # ---- the coor_descent loop: r_t = sum_j min(eS, r_{t-1}) ----
            es = [sb.tile([P, P * (m + 1)], bf, tag=f"es{m}", name=f"es{m}") for m in range(NT)]
            r = [sb.tile([P, N_ITERS + 1], f32, tag=f"r{m}", name=f"r{m}") for m in range(NT)]
            for m in range(NT):
                nc.vector.memset(r[m][:, 0:1], 1.0)
            for it in range(1, N_ITERS + 1):
                for m in range(NT):
                    W = P * (m + 1)
                    eng = nc.gpsimd if m in GP_TILES else nc.vector
                    eng.tensor_scalar(
                        es[m][:, :W], eS[m][:, :W], r[m][:, it - 1:it], None,
                        Alu.min, Alu.add, accum_out=r[m][:, it:it + 1])

            # ---- final attn = min(eS * (1/r), 1), in es scratch ----
            for m in range(NT):
                W = P * (m + 1)
                rec = sb.tile([P, 1], f32, tag=f"rec{m}", name=f"rec{m}")
                nc.vector.reciprocal(rec[:], r[m][:, N_ITERS:N_ITERS + 1])
                nc.vector.tensor_scalar(es[m][:, :W], eS[m][:, :W], rec[:], 1.0,
                                        Alu.mult, Alu.min)

            # ---- transpose attn -> attnT per j-chunk c: [128, N - 128c] ----
            aT = [sb.tile([P, N - P * c], bf, tag=f"aT{c}", name=f"aT{c}") for c in range(NT)]
            for m in range(NT):
                for c in range(m + 1):
                    tr = ptr.tile([P, P], bf, tag="tr")
                    nc.tensor.transpose(tr[:], es[m][:, P * c:P * (c + 1)], ident[:])
                    dst = aT[c][:, P * (m - c):P * (m - c + 1)]
                    if (m + c) % 2 == 0:
                        nc.scalar.copy(dst, tr[:])
                    else:
                        nc.vector.tensor_copy(dst, tr[:])

            # ---- outT_h[:, m-block] = sum_c v_c^T-contracted attnT -> [64, 8, 128] ----
            oT = sb.tile([DH, NT, P], bf, tag="oT")
            for m in range(NT):
                ps = pqk.tile([DH, P], f32, tag="pqk")
                for c in range(m + 1):
                    nc.tensor.matmul(ps[:], v_sb[c][:], aT[c][:, P * (m - c):P * (m - c + 1)],
                                     start=(c == 0), stop=(c == m))
                if m % 2 == 0:
                    nc.scalar.copy(oT[:, m, :], ps[:])
                else:
                    nc.vector.tensor_copy(oT[:, m, :], ps[:])

            # ---- AllToAll (bf16): shard j of core c = outT_c[:, 128j:128j+128] ----
            a2a_in = dram.tile([NT, DH, P], bf, tag="a2a_in")
            a2a_out = dram.tile([NT, DH, P], bf, tag="a2a_out")
            nc.sync.dma_start(a2a_in[:].rearrange("j p f -> p j f"), oT[:])
            nc.gpsimd.collective_compute(
                "AllToAll", Alu.bypass,
                replica_groups=[list(range(HEADS))],
                ins=[a2a_in.opt()], outs=[a2a_out.opt()])

            # ---- y rows for my token block: lhsT = outT_all [512, 128] ----
            oAll = sb.tile([P, KC, P], bf, tag="oAll")
            nc.sync.dma_start(oAll[:], a2a_out[:].rearrange("(kc g) p f -> (g p) kc f", g=2))
            yps = pmm.tile([P, DIM], f32, tag="psim")
            for kc in range(KC):
                nc.tensor.matmul(yps[:], oAll[:, kc, :], wo_sb[:, kc, :],
                                 start=(kc == 0), stop=(kc == KC - 1))
            y_sb = sb.tile([P, DIM], f32, tag="y")
            nc.scalar.copy(y_sb[:], yps[:])
            nc.sync.dma_start(out_ext[:], y_sb[:])

    nc.compile()
    return nc


def _prep_inputs(x, gamma, beta, w_qkv, w_out):
    x2 = np.ascontiguousarray(np.asarray(x, dtype=np.float32).reshape(N, DIM))
    gamma = np.asarray(gamma, dtype=np.float32)
    beta = np.asarray(beta, dtype=np.float32)
    w_qkv = np.asarray(w_qkv, dtype=np.float32)
    w_out = np.ascontiguousarray(np.asarray(w_out, dtype=np.float32))
    wfold = gamma[:, None] * w_qkv          # LN gamma folded into weights
    bfold = beta @ w_qkv                    # LN beta folded into bias
    in_maps = []
    for c in range(HEADS):
        qs = slice(c * DH, (c + 1) * DH)
        ks = slice(DIM + c * DH, DIM + (c + 1) * DH)
        vs = slice(2 * DIM + c * DH, 2 * DIM + (c + 1) * DH)
        in_maps.append({
            "x": x2,
            "wq": np.ascontiguousarray(wfold[:, qs] * QSCALE),
            "wk": np.ascontiguousarray(wfold[:, ks]),
            "wv": np.ascontiguousarray(wfold[:, vs]),
            "bq": np.ascontiguousarray((bfold[qs] * QSCALE)[:, None]),
            "bk": np.ascontiguousarray(bfold[ks][:, None]),
            "bv": np.ascontiguousarray(bfold[vs][None, :]),
            "wo": w_out,
        })
    return in_maps


def kernel(x, gamma, beta, w_qkv, w_out, _trace=False, **trace_kwargs):
    from concourse.bass_utils import run_bass_kernel_spmd

    if "nc" not in _cache:
        _cache["nc"] = _build()
    nc = _cache["nc"]
    in_maps = _prep_inputs(x, gamma, beta, w_qkv, w_out)
    res = run_bass_kernel_spmd(nc, in_maps, core_ids=list(range(HEADS)),
                               trace=_trace, **trace_kwargs)
    if _trace:
        _cache["last_result"] = res
    y = np.concatenate([res.results[c]["out"] for c in range(HEADS)], axis=0)
    return y.reshape(1, N, DIM)


# revision 7
# speedup vs baseline: 1.1978x; 1.0545x over previous
"""Distributed Trainium2 Bass kernel for sparse coor_descent attention.

Strategy: one head per NeuronCore (8 heads / 8 cores).
Key algebraic reformulation of coor_descent (k=1, constant=0):
    s+b = min(s, -a)  and exp is monotone, so with S = s/eps, eS = exp(S):
        r_{t} = sum_j min(eS_ij, r_{t-1,i}),   r_0 = 1
        attn  = min(eS / r_25, 1)
which runs as ONE fused DVE tensor_scalar(min, accum_out=sum) per row-tile
per iteration -- no transcendentals in the loop.

LN affine (gamma/beta) is folded into w_qkv on the host; the q scale and
the 1/eps are folded into the q-projection weights. Causal masking zeroes
eS above the diagonal (exp(-inf) = 0); strictly-upper blocks are never
stored or processed (triangular work-skipping).

Data path is bf16 (activations, weights, eS, attn); all accumulation
(PSUM, the r sums) stays f32. Validated rel err ~7e-3 vs the f32
reference (gate 2e-2).

Final projection: per-head output columns are exchanged via AllToAll so
core c ends with all heads' outputs for its token block, then computes
y rows [128c:128c+128] = outT_all^T @ w_out locally.
"""

import sys
import numpy as np

sys.path.insert(0, "/opt/trn_rl_repo")

HEADS = 8
DH = 64
DIM = 512
N = 1024
P = 128
NT = N // P  # 8 token row-tiles
KC = DIM // P  # 4 contraction chunks
EPS = 0.1
LN_EPS = 1e-5
N_ITERS = 25
QSCALE = (DH ** -0.5) / EPS  # fold head scale and 1/eps into q

# row-tiles of the coor_descent loop handled by the ACT engine via the
# relu-cancel identity sum_j min(eS,r) = W*r - sum_j relu(r - eS)
ACT_TILES = (5, 6, 7)

_cache = {}


def _build():
    from concourse import bacc, mybir
    import concourse.bass as bass
    import concourse.tile as tile
    from concourse.masks import make_identity

    f32 = mybir.dt.float32
    bf = mybir.dt.bfloat16
    Alu = mybir.AluOpType
    Act = mybir.ActivationFunctionType

    nc = bacc.Bacc("TRN2", target_bir_lowering=False, debug=False,
                   enable_asserts=True, num_devices=HEADS)

    x_ext = nc.dram_tensor("x", [N, DIM], f32, kind="ExternalInput")
    wq_ext = nc.dram_tensor("wq", [DIM, DH], f32, kind="ExternalInput")
    wk_ext = nc.dram_tensor("wk", [DIM, DH], f32, kind="ExternalInput")
    wv_ext = nc.dram_tensor("wv", [DIM, DH], f32, kind="ExternalInput")
    bq_ext = nc.dram_tensor("bq", [DH, 1], f32, kind="ExternalInput")
    bk_ext = nc.dram_tensor("bk", [DH, 1], f32, kind="ExternalInput")
    bv_ext = nc.dram_tensor("bv", [1, DH], f32, kind="ExternalInput")
    wo_ext = nc.dram_tensor("wo", [DIM, DIM], f32, kind="ExternalInput")
    out_ext = nc.dram_tensor("out", [P, DIM], f32, kind="ExternalOutput")

    with tile.TileContext(nc) as tc:
        with (
            tc.tile_pool(name="sb", bufs=1) as sb,
            tc.tile_pool(name="pmm", bufs=3, space="PSUM") as pmm,
            tc.tile_pool(name="pqk", bufs=2, space="PSUM") as pqk,
            tc.tile_pool(name="ptr", bufs=3, space="PSUM") as ptr,
            tc.tile_pool(name="dram", bufs=1, space="DRAM") as dram,
        ):
            ident = sb.tile([P, P], bf, tag="ident")
            make_identity(nc, ident[:])

            # ---- weight DMAs (f32) + on-chip converts to bf16 ----
            wq_f = sb.tile([P, KC, DH], f32, tag="wq_f")
            wk_f = sb.tile([P, KC, DH], f32, tag="wk_f")
            wv_f = sb.tile([P, KC, DH], f32, tag="wv_f")
            nc.sync.dma_start(wq_f[:], wq_ext[:].rearrange("(kc p) m -> p kc m", p=P))
            nc.sync.dma_start(wk_f[:], wk_ext[:].rearrange("(kc p) m -> p kc m", p=P))
            nc.sync.dma_start(wv_f[:], wv_ext[:].rearrange("(kc p) m -> p kc m", p=P))
            wq_sb = sb.tile([P, KC, DH], bf, tag="wq")
            wk_sb = sb.tile([P, KC, DH], bf, tag="wk")
            wv_sb = sb.tile([P, KC, DH], bf, tag="wv")
            nc.scalar.copy(wq_sb[:], wq_f[:])
            nc.scalar.copy(wk_sb[:], wk_f[:])
            nc.scalar.copy(wv_sb[:], wv_f[:])
            bq_sb = sb.tile([DH, 1], f32, tag="bq")
            bk_sb = sb.tile([DH, 1], f32, tag="bk")
            bv_f = sb.tile([1, DH], f32, tag="bv_f")
            nc.sync.dma_start(bq_sb[:], bq_ext[:])
            nc.sync.dma_start(bk_sb[:], bk_ext[:])
            nc.sync.dma_start(bv_f[:], bv_ext[:])
            bv_sb = sb.tile([1, DH], bf, tag="bv")
            nc.scalar.copy(bv_sb[:], bv_f[:])
            wo_f = sb.tile([P, KC, DIM], f32, tag="wo_f")
            nc.sync.dma_start(wo_f[:], wo_ext[:].rearrange("(kc p) e -> p kc e", p=P))
            wo_sb = sb.tile([P, KC, DIM], bf, tag="wo")
            nc.vector.tensor_copy(wo_sb[:], wo_f[:])
            ones_sb = sb.tile([1, P], bf, tag="ones")
            nc.vector.memset(ones_sb[:], 1.0)

            # ---- x DMA + LayerNorm (no affine; folded into weights) ----
            xin = [sb.tile([P, DIM], f32, tag=f"xin{t}", name=f"xin{t}") for t in range(NT)]
            xh = [sb.tile([P, DIM], bf, tag=f"xh{t}", name=f"xh{t}") for t in range(NT)]
            sq_scr = sb.tile([P, DIM], f32, tag="sq_scr")
            for t in range(NT):
                nc.sync.dma_start(xin[t][:], x_ext[P * t:P * (t + 1), :])
            for t in range(NT):
                stat = sb.tile([P, 6], f32, tag=f"stat{t}", name=f"stat{t}")
                # stat cols: 0=sum, 1=ssq, 2=mu, 3=bias(eps-mu^2), 4=std, 5=rstd
                nc.vector.tensor_reduce(stat[:, 0:1], xin[t][:], mybir.AxisListType.X, Alu.add)
                nc.scalar.activation(sq_scr[:], xin[t][:], Act.Square, accum_out=stat[:, 1:2])
                nc.scalar.mul(stat[:, 2:3], stat[:, 0:1], 1.0 / DIM)
                nc.scalar.square(stat[:, 3:4], stat[:, 2:3])
                nc.vector.tensor_scalar(stat[:, 3:4], stat[:, 3:4], -1.0, LN_EPS, Alu.mult, Alu.add)
                nc.scalar.activation(stat[:, 4:5], stat[:, 1:2], Act.Sqrt,
                                     bias=stat[:, 3:4], scale=1.0 / DIM)
                nc.vector.reciprocal(stat[:, 5:6], stat[:, 4:5])
                nc.vector.tensor_scalar(xh[t][:], xin[t][:], stat[:, 2:3], stat[:, 5:6],
                                        Alu.subtract, Alu.mult)

            # ---- transpose xh -> xhT [512, 1024] (4 tiles of [128, 1024]) ----
            xhT = [sb.tile([P, N], bf, tag=f"xhT{u}", name=f"xhT{u}") for u in range(KC)]
            for t in range(NT):
                for u in range(KC):
                    tr = ptr.tile([P, P], bf, tag="tr")
                    nc.tensor.transpose(tr[:], xh[t][:, P * u:P * (u + 1)], ident[:])
                    if (t + u) % 2 == 0:
                        nc.scalar.copy(xhT[u][:, P * t:P * (t + 1)], tr[:])
                    else:
                        nc.vector.tensor_copy(xhT[u][:, P * t:P * (t + 1)], tr[:])

            # ---- qT/kT = [64, 1024] bf16, v natural [128, 64] x 8 bf16 ----
            qT = sb.tile([DH, N], bf, tag="qT")
            kT = sb.tile([DH, N], bf, tag="kT")
            for dst_sb, w_sb, b_sb in ((qT, wq_sb, bq_sb), (kT, wk_sb, bk_sb)):
                for nb in range(2):
                    ps = pqk.tile([DH, 512], f32, tag="pqk")
                    for kc in range(KC):
                        nc.tensor.matmul(ps[:], w_sb[:, kc, :],
                                         xhT[kc][:, 512 * nb:512 * (nb + 1)],
                                         start=(kc == 0), stop=(kc == KC - 1))
                    nc.scalar.activation(dst_sb[:, 512 * nb:512 * (nb + 1)], ps[:],
                                         Act.Identity, bias=b_sb[:])
            v_sb = [sb.tile([P, DH], bf, tag=f"v{c}", name=f"v{c}") for c in range(NT)]
            for c in range(NT):
                ps = pqk.tile([P, DH], f32, tag="pqk")
                for kc in range(KC):
                    nc.tensor.matmul(ps[:], xhT[kc][:, P * c:P * (c + 1)], wv_sb[:, kc, :],
                                     start=(kc == 0), stop=False)
                nc.tensor.matmul(ps[:], ones_sb[:, 0:P], bv_sb[:], start=False, stop=True)
                if c % 2 == 0:
                    nc.scalar.copy(v_sb[c][:], ps[:])
                else:
                    nc.vector.tensor_copy(v_sb[c][:], ps[:])

            # ---- sim matmuls + fused exp: eS[m] = exp(qT_m^T @ kT), causal ----
            eS = [sb.tile([P, P * (m + 1)], bf, tag=f"eS{m}", name=f"eS{m}") for m in range(NT)]
            for m in range(NT):
                W = P * (m + 1)
                for nb in range((W + 511) // 512):
                    w = min(512, W - 512 * nb)
                    ps = pmm.tile([P, 512], f32, tag="psim")
                    nc.tensor.matmul(ps[:, :w], qT[:, P * m:P * (m + 1)],
                                     kT[:, 512 * nb:512 * nb + w])
                    nc.scalar.activation(eS[m][:, 512 * nb:512 * nb + w], ps[:, :w], Act.Exp)
                # causal mask on the diagonal block: keep j <= p, else 0
                nc.gpsimd.affine_select(
                    out=eS[m][:, W - P:W], in_=eS[m][:, W - P:W],
                    compare_op=Alu.is_ge, fill=0.0, base=0,
                    pattern=[[-1, P]], channel_multiplier=1)

            # ---- the coor_descent loop: r_t = sum_j min(eS, r_{t-1}) ----
            es = [sb.tile([P, P * (m + 1)], bf, tag=f"es{m}", name=f"es{m}") for m in range(NT)]
            esa = {m: sb.tile([P, P * (m + 1)], f32, tag=f"esa{m}", name=f"esa{m}")
                   for m in ACT_TILES}
            Tt = {m: sb.tile([P, N_ITERS + 1], f32, tag=f"T{m}", name=f"T{m}")
                  for m in ACT_TILES}
            r = [sb.tile([P, N_ITERS + 1], f32, tag=f"r{m}", name=f"r{m}") for m in range(NT)]
            for m in range(NT):
                nc.vector.memset(r[m][:, 0:1], 1.0)
            for it in range(1, N_ITERS + 1):
                for m in range(NT):
                    W = P * (m + 1)
                    if m in ACT_TILES:
                        # T = sum_j relu(r - eS);  r_new = W*r - T
                        nc.scalar.activation(
                            esa[m][:, :W], eS[m][:, :W], Act.Relu,
                            bias=r[m][:, it - 1:it], scale=-1.0,
                            accum_out=Tt[m][:, it:it + 1])
                        nc.vector.scalar_tensor_tensor(
                            r[m][:, it:it + 1], r[m][:, it - 1:it], float(W),
                            Tt[m][:, it:it + 1], Alu.mult, Alu.subtract)
                    else:
                        nc.vector.tensor_scalar(
                            es[m][:, :W], eS[m][:, :W], r[m][:, it - 1:it], None,
                            Alu.min, Alu.add, accum_out=r[m][:, it:it + 1])

            # ---- final attn = min(eS * (1/r), 1), in es scratch ----
            for m in range(NT):
                W = P * (m + 1)
                rec = sb.tile([P, 1], f32, tag=f"rec{m}", name=f"rec{m}")
                nc.vector.reciprocal(rec[:], r[m][:, N_ITERS:N_ITERS + 1])
                nc.vector.tensor_scalar(es[m][:, :W], eS[m][:, :W], rec[:], 1.0,
                                        Alu.mult, Alu.min)

            # ---- transpose attn -> attnT per j-chunk c: [128, N - 128c] ----
            aT = [sb.tile([P, N - P * c], bf, tag=f"aT{c}", name=f"aT{c}") for c in range(NT)]
            for m in range(NT):
                for c in range(m + 1):
                    tr = ptr.tile([P, P], bf, tag="tr")
                    nc.tensor.transpose(tr[:], es[m][:, P * c:P * (c + 1)], ident[:])
                    dst = aT[c][:, P * (m - c):P * (m - c + 1)]
                    if (m + c) % 2 == 0:
                        nc.scalar.copy(dst, tr[:])
                    else:
                        nc.vector.tensor_copy(dst, tr[:])

            # ---- outT_h[:, m-block] = sum_c v_c^T-contracted attnT -> [64, 8, 128] ----
            oT = sb.tile([DH, NT, P], bf, tag="oT")
            for m in range(NT):
                ps = pqk.tile([DH, P], f32, tag="pqk")
                for c in range(m + 1):
                    nc.tensor.matmul(ps[:], v_sb[c][:], aT[c][:, P * (m - c):P * (m - c + 1)],
                                     start=(c == 0), stop=(c == m))
                if m % 2 == 0:
                    nc.scalar.copy(oT[:, m, :], ps[:])
                else:
                    nc.vector.tensor_copy(oT[:, m, :], ps[:])

            # ---- AllToAll (bf16): shard j of core c = outT_c[:, 128j:128j+128] ----
            a2a_in = dram.tile([NT, DH, P], bf, tag="a2a_in")
            a2a_out = dram.tile([NT, DH, P], bf, tag="a2a_out")
            nc.sync.dma_start(a2a_in[:].rearrange("j p f -> p j f"), oT[:])
            nc.gpsimd.collective_compute(
                "AllToAll", Alu.bypass,
                replica_groups=[list(range(HEADS))],
                ins=[a2a_in.opt()], outs=[a2a_out.opt()])

            # ---- y rows for my token block: lhsT = outT_all [512, 128] ----
            oAll = sb.tile([P, KC, P], bf, tag="oAll")
            nc.sync.dma_start(oAll[:], a2a_out[:].rearrange("(kc g) p f -> (g p) kc f", g=2))
            yps = pmm.tile([P, DIM], f32, tag="psim")
            for kc in range(KC):
                nc.tensor.matmul(yps[:], oAll[:, kc, :], wo_sb[:, kc, :],
                                 start=(kc == 0), stop=(kc == KC - 1))
            y_sb = sb.tile([P, DIM], f32, tag="y")
            nc.scalar.copy(y_sb[:], yps[:])
            nc.sync.dma_start(out_ext[:], y_sb[:])

    nc.compile()
    return nc


def _prep_inputs(x, gamma, beta, w_qkv, w_out):
    x2 = np.ascontiguousarray(np.asarray(x, dtype=np.float32).reshape(N, DIM))
    gamma = np.asarray(gamma, dtype=np.float32)
    beta = np.asarray(beta, dtype=np.float32)
    w_qkv = np.asarray(w_qkv, dtype=np.float32)
    w_out = np.ascontiguousarray(np.asarray(w_out, dtype=np.float32))
    wfold = gamma[:, None] * w_qkv          # LN gamma folded into weights
    bfold = beta @ w_qkv                    # LN beta folded into bias
    in_maps = []
    for c in range(HEADS):
        qs = slice(c * DH, (c + 1) * DH)
        ks = slice(DIM + c * DH, DIM + (c + 1) * DH)
        vs = slice(2 * DIM + c * DH, 2 * DIM + (c + 1) * DH)
        in_maps.append({
            "x": x2,
            "wq": np.ascontiguousarray(wfold[:, qs] * QSCALE),
            "wk": np.ascontiguousarray(wfold[:, ks]),
            "wv": np.ascontiguousarray(wfold[:, vs]),
            "bq": np.ascontiguousarray((bfold[qs] * QSCALE)[:, None]),
            "bk": np.ascontiguousarray(bfold[ks][:, None]),
            "bv": np.ascontiguousarray(bfold[vs][None, :]),
            "wo": w_out,
        })
    return in_maps


def kernel(x, gamma, beta, w_qkv, w_out, _trace=False, **trace_kwargs):
    from concourse.bass_utils import run_bass_kernel_spmd

    if "nc" not in _cache:
        _cache["nc"] = _build()
    nc = _cache["nc"]
    in_maps = _prep_inputs(x, gamma, beta, w_qkv, w_out)
    res = run_bass_kernel_spmd(nc, in_maps, core_ids=list(range(HEADS)),
                               trace=_trace, **trace_kwargs)
    if _trace:
        _cache["last_result"] = res
    y = np.concatenate([res.results[c]["out"] for c in range(HEADS)], axis=0)
    return y.reshape(1, N, DIM)
